# revision 1
# baseline (speedup 1.0000x reference)
"""Cross-attention layer on 8 Trainium2 NeuronCores (Bass/Tile SPMD).

Sharding: tensor-parallel over heads. Each core owns 4 of the 32 heads:
it projects Q^T/K^T/V for its heads (bf16 matmuls, fp32 accumulate),
runs masked softmax attention in transposed layout (scores^T so the
softmax v-reduction is a PE ones-matmul and no attn transpose is ever
needed), then an AllToAll redistributes ctx^T from head-sharded to
token-sharded so every core runs the output projection + residual +
LayerNorm for its own 256-token slice. Host concatenates the 8 slices.

Numerics: matmul inputs bf16 (error ~1e-3 of output scale, validated
against the fp32 reference), all accumulation fp32, softmax without
max-subtraction (scores ~N(0,1), exp can't overflow), mask folded into
the exp bias, 1/sqrt(hd) folded into Wq on host, bv folded into an
effective bo on host (rows of attn sum to 1), residual+LN in fp32.
"""
import sys

sys.path.insert(0, "/opt/trn_rl_repo")

import numpy as np
import ml_dtypes

import concourse.bacc as bacc
import concourse.mybir as mybir
import concourse.tile as tile
from concourse.bass_utils import run_bass_kernel_spmd

BF16 = ml_dtypes.bfloat16

NCORES = 8
P = 128            # partitions / head dim / k-tile
H = 4096
KT = H // P        # 32 k-tiles along any H contraction
NH = 32
NHL = NH // NCORES  # 4 local heads
CW = NHL * P       # 512 local c-columns
B = 2
LB = 1024          # tokens per batch
L2 = B * LB        # 2048 total tokens
TL = L2 // NCORES  # 256 tokens per core after A2A
QW = 512           # token-quarter width in phase A
NQ = L2 // QW      # 4
NVT = L2 // P      # 16 v tiles total (8 per batch)
MSK = -1e30

_CACHE = {}

F32 = mybir.dt.float32
BF = mybir.dt.bfloat16


def _build(debug=False):
    nc = bacc.Bacc("TRN2", target_bir_lowering=False, debug=False,
                   num_devices=NCORES)

    hidT_d = nc.dram_tensor("hidT", [H, L2], BF, kind="ExternalInput")
    visT_d = nc.dram_tensor("visT", [H, L2], BF, kind="ExternalInput")
    wqT_d = nc.dram_tensor("wqT", [H, CW], BF, kind="ExternalInput")
    wkT_d = nc.dram_tensor("wkT", [H, CW], BF, kind="ExternalInput")
    wvT_d = nc.dram_tensor("wvT", [H, CW], BF, kind="ExternalInput")
    woT_d = nc.dram_tensor("woT", [H, H], BF, kind="ExternalInput")
    bqT_d = nc.dram_tensor("bqT", [P, NHL], F32, kind="ExternalInput")
    bkT_d = nc.dram_tensor("bkT", [P, NHL], F32, kind="ExternalInput")
    mskb_d = nc.dram_tensor("mskb", [P, B * 8], F32, kind="ExternalInput")
    hb_d = nc.dram_tensor("hb", [TL, H], F32, kind="ExternalInput")
    g_d = nc.dram_tensor("g", [P, H], F32, kind="ExternalInput")
    bta_d = nc.dram_tensor("bta", [P, H], F32, kind="ExternalInput")
    out_d = nc.dram_tensor("out", [TL, H], F32, kind="ExternalOutput")
    if debug:
        qT_dbg = nc.dram_tensor("qT_dbg", [P, NHL * L2], BF, kind="ExternalOutput")
        kT_dbg = nc.dram_tensor("kT_dbg", [P, NHL * L2], BF, kind="ExternalOutput")
        v_dbg = nc.dram_tensor("v_dbg", [P, NVT * CW], BF, kind="ExternalOutput")
        ctxT_dbg = nc.dram_tensor("ctxT_dbg", [P, NHL * L2], BF, kind="ExternalOutput")
        octxT_dbg = nc.dram_tensor("octxT_dbg", [P, KT * TL], BF, kind="ExternalOutput")
        xpre_dbg = nc.dram_tensor("xpre_dbg", [TL, H], F32, kind="ExternalOutput")

    with tile.TileContext(nc) as tc:
        with tc.tile_pool(name="persist", bufs=1) as pers, \
             tc.tile_pool(name="dram", bufs=1, space="DRAM") as dram:

            pqkv = tc.alloc_tile_pool(name="pqkv", bufs=1)
            qT_sb = pqkv.tile([P, NHL * L2], BF)     # Q^T/sqrt(hd): [hd, (h, l)]
            kT_sb = pqkv.tile([P, NHL * L2], BF)     # K^T: [hd, (h, v)]
            v_sb = pqkv.tile([P, NVT * CW], BF)      # V: [v, (vt, c)]
            ctxT_sb = pqkv.tile([P, NHL * L2], BF)   # ctx^T normalized: [hd, (h, l)]
            bqT_sb = pers.tile([P, NHL], F32)
            bkT_sb = pers.tile([P, NHL], F32)
            mskb_sb = pers.tile([P, B * 8], F32)
            ones_bf = pers.tile([P, 1], BF)
            ones_f32 = pers.tile([1, P], F32)
            nc.sync.dma_start(out=bqT_sb[:], in_=bqT_d[:])
            nc.sync.dma_start(out=bkT_sb[:], in_=bkT_d[:])
            nc.sync.dma_start(out=mskb_sb[:], in_=mskb_d[:])
            nc.vector.memset(ones_bf[:], 1.0)
            nc.vector.memset(ones_f32[:], 1.0)

            # ---------------- Phase A: Q^T, K^T, V projections ----------------
            with tc.tile_pool(name="phaseA", bufs=2) as pa, \
                 tc.tile_pool(name="psA", bufs=6, space="PSUM") as psA:

                def load_w(dram_t, tag="wproj"):
                    w_sb = pa.tile([P, KT * CW], BF, tag=tag, name="w_sb")
                    nc.sync.dma_start(
                        out=w_sb[:].rearrange("p (kt c) -> p kt c", kt=KT),
                        in_=dram_t[:].rearrange("(kt p) c -> p kt c", p=P))
                    return w_sb

                wq_sb = load_w(wqT_d)
                wk_sb = load_w(wkT_d)
                wv_sb = load_w(wvT_d)

                def proj_qk(x_dram, w_sb, b_sb, dst_sb):
                    for q in range(NQ):
                        xT = pa.tile([P, KT * QW], BF, tag="xT")
                        nc.sync.dma_start(
                            out=xT[:].rearrange("p (kt l) -> p kt l", kt=KT),
                            in_=x_dram[:, q * QW:(q + 1) * QW]
                                .rearrange("(kt p) l -> p kt l", p=P))
                        for h in range(NHL):
                            ps = psA.tile([P, QW], F32, tag="psA")
                            for kt in range(KT):
                                nc.tensor.matmul(
                                    ps[:],
                                    w_sb[:, kt * CW + h * P: kt * CW + (h + 1) * P],
                                    xT[:, kt * QW:(kt + 1) * QW],
                                    start=(kt == 0), stop=(kt == KT - 1))
                            nc.vector.tensor_scalar_add(
                                dst_sb[:, h * L2 + q * QW: h * L2 + (q + 1) * QW],
                                ps[:], b_sb[:, h:h + 1])

                proj_qk(hidT_d, wq_sb, bqT_sb, qT_sb)
                proj_qk(visT_d, wk_sb, bkT_sb, kT_sb)

                # V in natural [v, c] layout: lhsT = visT tile, rhs = WvT
                for q in range(NQ):
                    xT = pa.tile([P, KT * QW], BF, tag="xT")
                    nc.sync.dma_start(
                        out=xT[:].rearrange("p (kt l) -> p kt l", kt=KT),
                        in_=visT_d[:, q * QW:(q + 1) * QW]
                            .rearrange("(kt p) l -> p kt l", p=P))
                    for vt in range(4):
                        g_vt = q * 4 + vt
                        ps = psA.tile([P, CW], F32, tag="psA")
                        for kt in range(KT):
                            nc.tensor.matmul(
                                ps[:],
                                xT[:, kt * QW + vt * P: kt * QW + (vt + 1) * P],
                                wv_sb[:, kt * CW:(kt + 1) * CW],
                                start=(kt == 0), stop=(kt == KT - 1))
                        nc.scalar.copy(
                            out=v_sb[:, g_vt * CW:(g_vt + 1) * CW], in_=ps[:])

            # ---------------- Phase B: attention per (batch, head) ----------------
            with tc.tile_pool(name="phaseB", bufs=2) as pb, \
                 tc.tile_pool(name="psB", bufs=2, space="PSUM") as psB:
                for b in range(B):
                    for h in range(NHL):
                        for lh in range(2):
                            qcol = h * L2 + b * LB + lh * QW
                            attnT = pb.tile([P, 8 * QW], BF, tag="attnT", bufs=3)
                            rs_ps = psB.tile([1, QW], F32, tag="rs")
                            for vb in range(8):
                                sc_ps = psB.tile([P, QW], F32, tag="sc")
                                nc.tensor.matmul(
                                    sc_ps[:],
                                    kT_sb[:, h * L2 + b * LB + vb * P:
                                          h * L2 + b * LB + (vb + 1) * P],
                                    qT_sb[:, qcol: qcol + QW],
                                    start=True, stop=True)
                                mcol = b * 8 + vb
                                nc.scalar.activation(
                                    attnT[:, vb * QW:(vb + 1) * QW], sc_ps[:],
                                    mybir.ActivationFunctionType.Exp,
                                    bias=mskb_sb[:, mcol:mcol + 1], scale=1.0)
                                nc.tensor.matmul(
                                    rs_ps[:], ones_bf[:],
                                    attnT[:, vb * QW:(vb + 1) * QW],
                                    start=(vb == 0), stop=(vb == 7))
                            rcp_sb = pb.tile([1, QW], F32, tag="rcp")
                            nc.vector.reciprocal(rcp_sb[:], rs_ps[:])
                            rcp_ps = psB.tile([P, QW], F32, tag="rcpp")
                            nc.tensor.matmul(rcp_ps[:], ones_f32[:], rcp_sb[:],
                                             start=True, stop=True)
                            rcp_rep = pb.tile([P, QW], F32, tag="rcprep")
                            nc.scalar.copy(out=rcp_rep[:], in_=rcp_ps[:])
                            ctx_ps = psB.tile([P, QW], F32, tag="ctx")
                            for vb in range(8):
                                nc.tensor.matmul(
                                    ctx_ps[:],
                                    v_sb[:, (b * 8 + vb) * CW + h * P:
                                         (b * 8 + vb) * CW + (h + 1) * P],
                                    attnT[:, vb * QW:(vb + 1) * QW],
                                    start=(vb == 0), stop=(vb == 7))
                            nc.vector.tensor_tensor(
                                out=ctxT_sb[:, qcol: qcol + QW],
                                in0=ctx_ps[:], in1=rcp_rep[:],
                                op=mybir.AluOpType.mult)

            if debug:
                nc.sync.dma_start(out=qT_dbg[:], in_=qT_sb[:])
                nc.sync.dma_start(out=kT_dbg[:], in_=kT_sb[:])
                nc.sync.dma_start(out=v_dbg[:], in_=v_sb[:])
                nc.sync.dma_start(out=ctxT_dbg[:], in_=ctxT_sb[:])

            # ---------------- Phase C: A2A, O-proj, residual + LN ----------------
            a2a_in = dram.tile([NCORES, CW, TL], BF)
            a2a_out = dram.tile([NCORES, CW, TL], BF)
            for h in range(NHL):
                nc.sync.dma_start(
                    out=a2a_in[:, h * P:(h + 1) * P, :]
                        .rearrange("j p l -> p j l"),
                    in_=ctxT_sb[:, h * L2:(h + 1) * L2]
                        .rearrange("p (j l) -> p j l", j=NCORES))
            nc.gpsimd.collective_compute(
                "AllToAll", mybir.AluOpType.bypass,
                replica_groups=[list(range(NCORES))],
                ins=[a2a_in[:]], outs=[a2a_out[:]])
            pqkv.release()

            with tc.tile_pool(name="phaseC", bufs=2) as pc, \
                 tc.tile_pool(name="psC", bufs=4, space="PSUM") as psC, \
                 tc.tile_pool(name="phaseC1", bufs=1) as pc1:
                octxT = pc1.tile([P, KT * TL], BF)  # [hd, (i, ct, l)] = full ctx^T cols
                for i in range(NCORES):
                    nc.sync.dma_start(
                        out=octxT[:, i * (NHL * TL):(i + 1) * (NHL * TL)]
                            .rearrange("p (ct l) -> p ct l", ct=NHL),
                        in_=a2a_out[i].rearrange("(ct p) l -> p ct l", p=P))

                if debug:
                    nc.sync.dma_start(out=octxT_dbg[:], in_=octxT[:])
                g_sb = pc1.tile([P, H], F32)
                nc.sync.dma_start(out=g_sb[:], in_=g_d[:])
                bta_sb = pc1.tile([P, H], F32)
                nc.sync.dma_start(out=bta_sb[:], in_=bta_d[:])
                x_sb = [pc1.tile([P, H], F32, name=f"x_sb{lt}") for lt in range(2)]

                MCW = 512
                for mc in range(H // MCW):
                    wo_sb = pc.tile([P, KT * MCW], BF, tag="wo")
                    nc.sync.dma_start(
                        out=wo_sb[:].rearrange("p (ct m) -> p ct m", ct=KT),
                        in_=woT_d[:, mc * MCW:(mc + 1) * MCW]
                            .rearrange("(ct p) m -> p ct m", p=P))
                    for lt in range(2):
                        po = psC.tile([P, MCW], F32, tag="po")
                        for g in range(KT):
                            nc.tensor.matmul(
                                po[:],
                                octxT[:, g * TL + lt * P: g * TL + (lt + 1) * P],
                                wo_sb[:, g * MCW:(g + 1) * MCW],
                                start=(g == 0), stop=(g == KT - 1))
                        nc.scalar.copy(
                            out=x_sb[lt][:, mc * MCW:(mc + 1) * MCW], in_=po[:])

                for lt in range(2):
                    if debug:
                        nc.sync.dma_start(
                            out=xpre_dbg[lt * P:(lt + 1) * P, :],
                            in_=x_sb[lt][:])
                    hb_sb = pc.tile([P, H], F32, tag="hb")
                    nc.sync.dma_start(out=hb_sb[:],
                                      in_=hb_d[lt * P:(lt + 1) * P, :])
                    x = x_sb[lt]
                    musum = pc.tile([P, 1], F32, tag="musum")
                    nc.vector.scalar_tensor_tensor(
                        out=x[:], in0=x[:], scalar=1.0, in1=hb_sb[:],
                        op0=mybir.AluOpType.mult, op1=mybir.AluOpType.add,
                        accum_out=musum[:])
                    mu = pc.tile([P, 1], F32, tag="mu")
                    nc.scalar.mul(mu[:], musum[:], 1.0 / H)
                    nc.vector.tensor_scalar(
                        out=x[:], in0=x[:], scalar1=mu[:], scalar2=None,
                        op0=mybir.AluOpType.subtract)
                    ssq = pc.tile([P, 1], F32, tag="ssq")
                    nc.scalar.activation(
                        hb_sb[:], x[:], mybir.ActivationFunctionType.Square,
                        accum_out=ssq[:])
                    eps_sb = pc.tile([P, 1], F32, tag="eps")
                    nc.vector.memset(eps_sb[:], 1e-5)
                    std = pc.tile([P, 1], F32, tag="std")
                    nc.scalar.activation(
                        std[:], ssq[:], mybir.ActivationFunctionType.Sqrt,
                        bias=eps_sb[:], scale=1.0 / H)
                    rstd = pc.tile([P, 1], F32, tag="rstd")
                    nc.vector.reciprocal(rstd[:], std[:])
                    o_sb = pc.tile([P, H], F32, tag="hb")
                    nc.vector.scalar_tensor_tensor(
                        out=o_sb[:], in0=x[:], scalar=rstd[:], in1=g_sb[:],
                        op0=mybir.AluOpType.mult, op1=mybir.AluOpType.mult)
                    nc.vector.tensor_tensor(
                        out=o_sb[:], in0=o_sb[:], in1=bta_sb[:],
                        op=mybir.AluOpType.add)
                    nc.sync.dma_start(out=out_d[lt * P:(lt + 1) * P, :],
                                      in_=o_sb[:])

    nc.compile()
    return nc


def _prep_inputs(hidden_states, vision_features, attention_mask,
                 Wq, bq, Wk, bk, Wv, bv, Wo, bo, ln_g, ln_b):
    f = np.asarray
    hs = f(hidden_states, dtype=np.float32).reshape(L2, H)
    vf = f(vision_features, dtype=np.float32).reshape(L2, H)
    am = f(attention_mask)
    Wq, bq = f(Wq, dtype=np.float32), f(bq, dtype=np.float32)
    Wk, bk = f(Wk, dtype=np.float32), f(bk, dtype=np.float32)
    Wv, bv = f(Wv, dtype=np.float32), f(bv, dtype=np.float32)
    Wo, bo = f(Wo, dtype=np.float32), f(bo, dtype=np.float32)
    ln_g, ln_b = f(ln_g, dtype=np.float32), f(ln_b, dtype=np.float32)

    s = 1.0 / np.sqrt(P)
    hidT = np.ascontiguousarray(hs.T).astype(BF16)
    visT = np.ascontiguousarray(vf.T).astype(BF16)
    woT = np.ascontiguousarray(Wo.T).astype(BF16)
    mb = np.where(am != 0, 0.0, MSK).astype(np.float32)          # (B, LB)
    mskb = np.ascontiguousarray(
        mb.reshape(B, 8, P).transpose(2, 0, 1).reshape(P, B * 8))
    bo_eff = bo + Wo @ bv
    g_rep = np.ascontiguousarray(np.broadcast_to(ln_g, (P, H)))
    b_rep = np.ascontiguousarray(np.broadcast_to(ln_b, (P, H)))

    in_maps = []
    for c in range(NCORES):
        sl = slice(c * CW, (c + 1) * CW)
        in_maps.append({
            "hidT": hidT,
            "visT": visT,
            "wqT": np.ascontiguousarray((Wq[sl] * s).T).astype(BF16),
            "wkT": np.ascontiguousarray(Wk[sl].T).astype(BF16),
            "wvT": np.ascontiguousarray(Wv[sl].T).astype(BF16),
            "woT": woT,
            "bqT": np.ascontiguousarray((bq[sl] * s).reshape(NHL, P).T),
            "bkT": np.ascontiguousarray(bk[sl].reshape(NHL, P).T),
            "mskb": mskb,
            "hb": np.ascontiguousarray(hs[c * TL:(c + 1) * TL] + bo_eff),
            "g": g_rep,
            "bta": b_rep,
        })
    return in_maps


def kernel(**inputs) -> np.ndarray:
    key = "dbg" if inputs.pop("_debug", False) else "main"
    if key not in _CACHE:
        _CACHE[key] = _build(debug=(key == "dbg"))
    nc = _CACHE[key]
    in_maps = _prep_inputs(**inputs)
    res = run_bass_kernel_spmd(nc, in_maps, list(range(NCORES)))
    out = np.concatenate([res.results[c]["out"] for c in range(NCORES)], axis=0)
    if key == "dbg":
        kernel._dbg = res.results
    return out.reshape(B, LB, H)



# revision 11
# speedup vs baseline: 1.6617x; 1.6617x over previous
"""Cross-attention layer on 8 Trainium2 NeuronCores (Bass/Tile SPMD).

Sharding: tensor-parallel over heads. Each core projects Q^T/K^T/V for its
4 heads (fp8e4 DoubleRow matmuls, fp32 accumulate, weights pre-scaled x64 on
host so they clear the fp8 subnormal range; descale folded into the PSUM
evacuation), runs masked softmax attention in bf16 transposed layout, then a
per-batch AllToAll redistributes ctx^T (fp8, x8 scaled) from head-sharded to
token-sharded so every core runs the fp8 output projection + residual +
LayerNorm for its 128-token slice of each batch. The batch split pipelines
attention(b1) under A2A(b0) and O-proj(b0) under A2A(b1).

Row-sums for softmax: DVE chunk-sum + one ones-matmul + one broadcast
matmul per (b,h,lh) instead of 9 tiny PE matmuls.
"""
import sys

sys.path.insert(0, "/opt/trn_rl_repo")

import numpy as np
import ml_dtypes

import concourse.bacc as bacc
import concourse.mybir as mybir
import concourse.tile as tile
from concourse.bass_utils import run_bass_kernel_spmd

BF16 = ml_dtypes.bfloat16
FP8 = ml_dtypes.float8_e4m3   # matches mybir.dt.float8e4 (max 240)

NCORES = 8
P = 128            # partitions / head dim
H = 4096
KT = H // P        # 32 k-tiles along any H contraction
NT = KT // 2       # 16 DoubleRow k-pair steps
NH = 32
NHL = NH // NCORES  # 4 local heads
CW = NHL * P       # 512 local c-columns
B = 2
LB = 1024          # tokens per batch
L2 = B * LB        # 2048 total tokens
TLB = LB // NCORES  # 128 tokens per core per batch
QW = 512           # token-block width in phase A
NQ = L2 // QW      # 4
MSK = -1e30
WS = 64.0          # fp8 weight pre-scale
CS = 8.0           # fp8 ctx pre-scale
SQ = 1.0 / np.sqrt(P)

_CACHE = {}

F32 = mybir.dt.float32
BF = mybir.dt.bfloat16
F8 = mybir.dt.float8e4
DR = mybir.MatmulPerfMode.DoubleRow


def _build(debug=False):
    nc = bacc.Bacc("TRN2", target_bir_lowering=False, debug=False,
                   num_devices=NCORES)

    hid_d = nc.dram_tensor("hid8", [P, NQ, KT, QW], F8, kind="ExternalInput")
    vis_d = nc.dram_tensor("vis8", [P, NQ, KT, QW], F8, kind="ExternalInput")
    wq_d = nc.dram_tensor("wq8", [P, KT, CW], F8, kind="ExternalInput")
    wk_d = nc.dram_tensor("wk8", [P, KT, CW], F8, kind="ExternalInput")
    wv_d = nc.dram_tensor("wv8", [P, KT, CW], F8, kind="ExternalInput")
    wo_d = nc.dram_tensor("wo8", [P, H // QW, KT, QW], F8, kind="ExternalInput")
    bqT_d = nc.dram_tensor("bqT", [P, NHL], F32, kind="ExternalInput")
    bkT_d = nc.dram_tensor("bkT", [P, NHL], F32, kind="ExternalInput")
    mskb_d = nc.dram_tensor("mskb", [P, B * 8], F32, kind="ExternalInput")
    hb_d = nc.dram_tensor("hb", [B * TLB, H], F32, kind="ExternalInput")
    g_d = nc.dram_tensor("g", [P, H], F32, kind="ExternalInput")
    bta_d = nc.dram_tensor("bta", [P, H], F32, kind="ExternalInput")
    out_d = nc.dram_tensor("out", [B * TLB, H], F32, kind="ExternalOutput")
    if debug:
        qT_dbg = nc.dram_tensor("qT_dbg", [P, NHL * L2], BF, kind="ExternalOutput")
        kT_dbg = nc.dram_tensor("kT_dbg", [P, NHL * L2], BF, kind="ExternalOutput")
        v_dbg = nc.dram_tensor("v_dbg", [P, 16 * CW], BF, kind="ExternalOutput")
        ctxT_dbg = nc.dram_tensor("ctxT_dbg", [P, NHL * L2], F32, kind="ExternalOutput")
        octxT_dbg = nc.dram_tensor("octxT_dbg", [P, B * KT * TLB], F32,
                                   kind="ExternalOutput")
        xpre_dbg = nc.dram_tensor("xpre_dbg", [B * TLB, H], F32, kind="ExternalOutput")

    with tile.TileContext(nc) as tc:
        with tc.tile_pool(name="persist", bufs=1) as pers, \
             tc.tile_pool(name="dram", bufs=1, space="DRAM") as dram:

            pqkv = tc.alloc_tile_pool(name="pqkv", bufs=1)
            qT_sb = pqkv.tile([P, NHL, L2], BF)     # Q^T/sqrt(hd): [hd, h, l]
            kT_sb = pqkv.tile([P, NHL, L2], BF)     # K^T: [hd, h, v]
            v_sb = pqkv.tile([P, 16, CW], BF)       # V: [v, vt, c]
            ctxT_sb = pqkv.tile([P, NHL, L2], F8)   # ctx^T * CS: [hd, h, l]
            bqT_sb = pers.tile([P, NHL], F32)
            bkT_sb = pers.tile([P, NHL], F32)
            mskb_sb = pers.tile([P, B * 8], F32)
            ones_bf = pers.tile([P, 1], BF)
            ones_f32 = pers.tile([1, P], F32)
            nc.sync.dma_start(out=bqT_sb[:], in_=bqT_d[:])
            nc.sync.dma_start(out=bkT_sb[:], in_=bkT_d[:])
            nc.sync.dma_start(out=mskb_sb[:], in_=mskb_d[:])
            nc.vector.memset(ones_bf[:], 1.0)
            nc.vector.memset(ones_f32[:], 1.0)

            # ---------------- Phase A: Q^T, K^T, V projections (fp8 DR) ----
            pa = tc.alloc_tile_pool(name="phaseA", bufs=1)
            with tc.tile_pool(name="psA", bufs=6, space="PSUM") as psA:
                wq_sb = pa.tile([P, KT, CW], F8, name="wq_sb")
                wk_sb = pa.tile([P, KT, CW], F8, name="wk_sb")
                wv_sb = pa.tile([P, KT, CW], F8, name="wv_sb")
                nc.sync.dma_start(out=wq_sb[:], in_=wq_d[:])

                def qk_block(xT, w_sb, b_sb, dst_sb, q, scale):
                    for h in range(NHL):
                        ps = psA.tile([P, QW], F32, tag="psA")
                        for t in range(NT):
                            nc.tensor.matmul(
                                ps[:],
                                w_sb[:, 2 * t:2 * t + 2, h * P:(h + 1) * P],
                                xT[:, 2 * t:2 * t + 2, :],
                                start=(t == 0), stop=(t == NT - 1),
                                perf_mode=DR)
                        nc.vector.tensor_scalar(
                            out=dst_sb[:, h, q * QW:(q + 1) * QW], in0=ps[:],
                            scalar1=scale, scalar2=b_sb[:, h:h + 1],
                            op0=mybir.AluOpType.mult, op1=mybir.AluOpType.add)

                # Q over hid blocks
                for q in range(NQ):
                    xT = pa.tile([P, KT, QW], F8, tag="xT", bufs=3)
                    nc.sync.dma_start(out=xT[:], in_=hid_d[:, q])
                    if q == 0:
                        nc.sync.dma_start(out=wk_sb[:], in_=wk_d[:])
                        nc.sync.dma_start(out=wv_sb[:], in_=wv_d[:])
                    qk_block(xT, wq_sb, bqT_sb, qT_sb, q, SQ / WS)

                # K and V share the vis block loads
                for q in range(NQ):
                    xT = pa.tile([P, KT, QW], F8, tag="xT", bufs=3)
                    nc.sync.dma_start(out=xT[:], in_=vis_d[:, q])
                    qk_block(xT, wk_sb, bkT_sb, kT_sb, q, 1.0 / WS)
                    for vt in range(4):
                        g_vt = q * 4 + vt
                        ps = psA.tile([P, CW], F32, tag="psA")
                        for t in range(NT):
                            nc.tensor.matmul(
                                ps[:],
                                xT[:, 2 * t:2 * t + 2, vt * P:(vt + 1) * P],
                                wv_sb[:, 2 * t:2 * t + 2, :],
                                start=(t == 0), stop=(t == NT - 1),
                                perf_mode=DR)
                        nc.scalar.activation(
                            v_sb[:, g_vt, :], ps[:],
                            mybir.ActivationFunctionType.Copy, scale=1.0 / WS)
            pa.release()

            # ------------- Phase B + C pipelined per batch -----------------
            pb = tc.alloc_tile_pool(name="phaseB", bufs=1)
            pc = tc.alloc_tile_pool(name="phaseC", bufs=1)
            ps = tc.alloc_tile_pool(name="psBC", bufs=1, space="PSUM")

            a2a_in = [dram.tile([NCORES, P, NHL, TLB], F8, name=f"a2a_in{i}")
                      for i in range(B)]
            a2a_out = [dram.tile([NCORES, P, NHL, TLB], F8, name=f"a2a_out{i}")
                       for i in range(B)]
            octxT = [None, None]

            def phaseB(b):
                for h in range(NHL):
                    for lh in range(2):
                        qoff = b * LB + lh * QW
                        attnT = pb.tile([P, 8, QW], BF, tag="attnT", bufs=2)
                        for vb in range(8):
                            sc_ps = ps.tile([P, QW], F32, tag="sc", bufs=2)
                            nc.tensor.matmul(
                                sc_ps[:],
                                kT_sb[:, h, b * LB + vb * P: b * LB + (vb + 1) * P],
                                qT_sb[:, h, qoff: qoff + QW],
                                start=True, stop=True)
                            nc.scalar.activation(
                                attnT[:, vb, :], sc_ps[:],
                                mybir.ActivationFunctionType.Exp,
                                bias=mskb_sb[:, b * 8 + vb: b * 8 + vb + 1],
                                scale=1.0)
                        # chunk-sum on DVE (bf16): S = sum_vb attnT[:, vb]
                        S = pb.tile([P, QW], BF, tag="S", bufs=2)
                        nc.vector.tensor_tensor(
                            out=S[:], in0=attnT[:, 0, :], in1=attnT[:, 1, :],
                            op=mybir.AluOpType.add)
                        for vb in range(2, 8):
                            nc.vector.tensor_tensor(
                                out=S[:], in0=S[:], in1=attnT[:, vb, :],
                                op=mybir.AluOpType.add)
                        ctx_ps = ps.tile([P, QW], F32, tag="ctx", bufs=2)
                        for vb in range(8):
                            nc.tensor.matmul(
                                ctx_ps[:],
                                v_sb[:, b * 8 + vb, h * P:(h + 1) * P],
                                attnT[:, vb, :],
                                start=(vb == 0), stop=(vb == 7))
                        # cross-partition row-sum + reciprocal + broadcast
                        rs_ps = ps.tile([1, QW], F32, tag="rs", bufs=1)
                        nc.tensor.matmul(rs_ps[:], ones_bf[:], S[:],
                                         start=True, stop=True)
                        rcp_sb = pb.tile([1, QW], F32, tag="rcp", bufs=2)
                        nc.vector.reciprocal(rcp_sb[:], rs_ps[:])
                        rcp_ps = ps.tile([P, QW], F32, tag="rcpp", bufs=1)
                        nc.tensor.matmul(rcp_ps[:], ones_f32[:], rcp_sb[:],
                                         start=True, stop=True)
                        rcp_rep = pb.tile([P, QW], F32, tag="rcprep", bufs=2)
                        nc.scalar.copy(out=rcp_rep[:], in_=rcp_ps[:])
                        # normalize * CS -> fp8 ctx^T
                        nc.vector.scalar_tensor_tensor(
                            out=ctxT_sb[:, h, qoff: qoff + QW],
                            in0=ctx_ps[:], scalar=CS, in1=rcp_rep[:],
                            op0=mybir.AluOpType.mult, op1=mybir.AluOpType.mult)

            def a2a(b):
                for h in range(NHL):
                    nc.sync.dma_start(
                        out=a2a_in[b][:, :, h, :].rearrange("j p l -> p j l"),
                        in_=ctxT_sb[:, h, b * LB:(b + 1) * LB]
                            .rearrange("p (j l) -> p j l", j=NCORES))
                nc.gpsimd.collective_compute(
                    "AllToAll", mybir.AluOpType.bypass,
                    replica_groups=[list(range(NCORES))],
                    ins=[a2a_in[b][:]], outs=[a2a_out[b][:]])
                octxT[b] = pc.tile([P, KT, TLB], F8, name=f"octxT{b}")
                for i in range(NCORES):
                    nc.sync.dma_start(
                        out=octxT[b][:, i * NHL:(i + 1) * NHL, :],
                        in_=a2a_out[b][i])

            def phaseC(b, g_sb, bta_sb):
                # O-projection for this batch's 128 tokens, all H columns
                hb_sb = pc.tile([P, H], F32, tag="hb", bufs=1)
                nc.sync.dma_start(out=hb_sb[:], in_=hb_d[b * TLB:(b + 1) * TLB, :])
                x = pc.tile([P, H], F32, tag="x", bufs=2)
                msum = pc.tile([P, 8], F32, tag="msum", bufs=2)
                for mc in range(H // QW):
                    wo_sb = pc.tile([P, KT, QW], F8, tag="wo", bufs=2)
                    nc.sync.dma_start(out=wo_sb[:], in_=wo_d[:, mc])
                    po = ps.tile([P, QW], F32, tag="po", bufs=2)
                    for t in range(NT):
                        nc.tensor.matmul(
                            po[:],
                            octxT[b][:, 2 * t:2 * t + 2, :],
                            wo_sb[:, 2 * t:2 * t + 2, :],
                            start=(t == 0), stop=(t == NT - 1),
                            perf_mode=DR)
                    # x = po/(WS*CS) + hb, with per-chunk row-sum accumulation
                    nc.vector.scalar_tensor_tensor(
                        out=x[:, mc * QW:(mc + 1) * QW],
                        in0=po[:], scalar=1.0 / (WS * CS),
                        in1=hb_sb[:, mc * QW:(mc + 1) * QW],
                        op0=mybir.AluOpType.mult, op1=mybir.AluOpType.add,
                        accum_out=msum[:, mc:mc + 1])
                if debug:
                    nc.sync.dma_start(
                        out=xpre_dbg[b * TLB:(b + 1) * TLB, :], in_=x[:])
                # ---- LayerNorm over H for the 128 tokens ----
                musum = pc.tile([P, 1], F32, tag="musum", bufs=2)
                nc.scalar.activation(
                    msum[:], msum[:], mybir.ActivationFunctionType.Copy,
                    accum_out=musum[:])
                mu = pc.tile([P, 1], F32, tag="mu", bufs=2)
                nc.scalar.mul(mu[:], musum[:], 1.0 / H)
                # Square scratch into hb_sb (already consumed by the evacs)
                ssq = pc.tile([P, 1], F32, tag="ssq", bufs=2)
                nc.scalar.activation(
                    hb_sb[:], x[:], mybir.ActivationFunctionType.Square,
                    accum_out=ssq[:])
                mu2 = pc.tile([P, 1], F32, tag="mu2", bufs=2)
                nc.scalar.activation(mu2[:], mu[:],
                                     mybir.ActivationFunctionType.Square)
                var = pc.tile([P, 1], F32, tag="var", bufs=2)
                nc.vector.scalar_tensor_tensor(
                    out=var[:], in0=ssq[:], scalar=1.0 / H, in1=mu2[:],
                    op0=mybir.AluOpType.mult, op1=mybir.AluOpType.subtract)
                eps_sb = pc.tile([P, 1], F32, tag="eps", bufs=1)
                nc.vector.memset(eps_sb[:], 1e-5)
                std = pc.tile([P, 1], F32, tag="std", bufs=2)
                nc.scalar.activation(std[:], var[:],
                                     mybir.ActivationFunctionType.Sqrt,
                                     bias=eps_sb[:], scale=1.0)
                rstd = pc.tile([P, 1], F32, tag="rstd", bufs=2)
                nc.vector.reciprocal(rstd[:], std[:])
                # normalize + gain + bias, in place on x
                nc.vector.tensor_scalar(
                    out=x[:], in0=x[:], scalar1=mu[:], scalar2=rstd[:],
                    op0=mybir.AluOpType.subtract, op1=mybir.AluOpType.mult)
                nc.vector.tensor_tensor(
                    out=x[:], in0=x[:], in1=g_sb[:], op=mybir.AluOpType.mult)
                nc.vector.tensor_tensor(
                    out=x[:], in0=x[:], in1=bta_sb[:], op=mybir.AluOpType.add)
                nc.sync.dma_start(out=out_d[b * TLB:(b + 1) * TLB, :],
                                  in_=x[:])

            phaseB(0)
            a2a(0)
            phaseB(1)
            if debug:
                nc.sync.dma_start(
                    out=qT_dbg[:], in_=qT_sb[:].rearrange("p h l -> p (h l)"))
                nc.sync.dma_start(
                    out=kT_dbg[:], in_=kT_sb[:].rearrange("p h l -> p (h l)"))
                nc.sync.dma_start(
                    out=v_dbg[:], in_=v_sb[:].rearrange("p t c -> p (t c)"))
                for h in range(NHL):
                    ctmp = pers.tile([P, L2], F32, tag="ctmp", bufs=2)
                    nc.scalar.copy(out=ctmp[:], in_=ctxT_sb[:, h, :])
                    nc.sync.dma_start(
                        out=ctxT_dbg[:, h * L2:(h + 1) * L2], in_=ctmp[:])
            g_sb = pc.tile([P, H], F32, name="g_sb")
            nc.sync.dma_start(out=g_sb[:], in_=g_d[:])
            bta_sb = pc.tile([P, H], F32, name="bta_sb")
            nc.sync.dma_start(out=bta_sb[:], in_=bta_d[:])
            phaseC(0, g_sb, bta_sb)
            a2a(1)
            if debug:
                for b in range(B):
                    for kq in range(4):
                        otmp = pers.tile([P, 8 * TLB], F32, tag="otmp", bufs=2)
                        nc.scalar.copy(
                            out=otmp[:],
                            in_=octxT[b][:, kq * 8:(kq + 1) * 8, :]
                                .rearrange("p k l -> p (k l)"))
                        nc.sync.dma_start(
                            out=octxT_dbg[:, (b * KT + kq * 8) * TLB:
                                          (b * KT + (kq + 1) * 8) * TLB],
                            in_=otmp[:])
            phaseC(1, g_sb, bta_sb)
            ps.release()
            pc.release()
            pb.release()
            pqkv.release()

    nc.compile()
    return nc


def _prep_inputs(hidden_states, vision_features, attention_mask,
                 Wq, bq, Wk, bk, Wv, bv, Wo, bo, ln_g, ln_b):
    f = np.asarray
    hs = f(hidden_states, dtype=np.float32).reshape(L2, H)
    vf = f(vision_features, dtype=np.float32).reshape(L2, H)
    am = f(attention_mask)
    Wq, bq = f(Wq, dtype=np.float32), f(bq, dtype=np.float32)
    Wk, bk = f(Wk, dtype=np.float32), f(bk, dtype=np.float32)
    Wv, bv = f(Wv, dtype=np.float32), f(bv, dtype=np.float32)
    Wo, bo = f(Wo, dtype=np.float32), f(bo, dtype=np.float32)
    ln_g, ln_b = f(ln_g, dtype=np.float32), f(ln_b, dtype=np.float32)

    def act_layout(x):  # [L2, H] -> [P, NQ, KT, QW] fp8
        return np.ascontiguousarray(
            x.T.reshape(KT, P, NQ, QW).transpose(1, 2, 0, 3)).astype(FP8)

    def w_layout(w_slice):  # [CW, H] -> [P, KT, CW] fp8, pre-scaled
        return np.ascontiguousarray(
            (w_slice.T * WS).reshape(KT, P, CW).transpose(1, 0, 2)).astype(FP8)

    hid8 = act_layout(hs)
    vis8 = act_layout(vf)
    # Wo full: [H(c), H(m)] -> [P, H//QW, KT, QW]
    wo8 = np.ascontiguousarray(
        (Wo.T * WS).reshape(KT, P, H // QW, QW).transpose(1, 2, 0, 3)).astype(FP8)
    mb = np.where(am != 0, 0.0, MSK).astype(np.float32)          # (B, LB)
    mskb = np.ascontiguousarray(
        mb.reshape(B, 8, P).transpose(2, 0, 1).reshape(P, B * 8))
    bo_eff = bo + Wo @ bv
    g_rep = np.ascontiguousarray(np.broadcast_to(ln_g, (P, H)))
    b_rep = np.ascontiguousarray(np.broadcast_to(ln_b, (P, H)))

    in_maps = []
    for c in range(NCORES):
        sl = slice(c * CW, (c + 1) * CW)
        hb = np.empty((B * TLB, H), np.float32)
        for b in range(B):
            rows = hs[b * LB + c * TLB:b * LB + (c + 1) * TLB]
            hb[b * TLB:(b + 1) * TLB] = rows + bo_eff
        in_maps.append({
            "hid8": hid8,
            "vis8": vis8,
            "wq8": w_layout(Wq[sl]),
            "wk8": w_layout(Wk[sl]),
            "wv8": w_layout(Wv[sl]),
            "wo8": wo8,
            "bqT": np.ascontiguousarray((bq[sl] * SQ).reshape(NHL, P).T),
            "bkT": np.ascontiguousarray(bk[sl].reshape(NHL, P).T),
            "mskb": mskb,
            "hb": hb,
            "g": g_rep,
            "bta": b_rep,
        })
    return in_maps


def kernel(**inputs) -> np.ndarray:
    key = "dbg" if inputs.pop("_debug", False) else "main"
    if key not in _CACHE:
        _CACHE[key] = _build(debug=(key == "dbg"))
    nc = _CACHE[key]
    in_maps = _prep_inputs(**inputs)
    res = run_bass_kernel_spmd(nc, in_maps, list(range(NCORES)))
    out = np.empty((B, LB, H), np.float32)
    for c in range(NCORES):
        o = res.results[c]["out"]
        for b in range(B):
            out[b, c * TLB:(c + 1) * TLB] = o[b * TLB:(b + 1) * TLB]
    if key == "dbg":
        kernel._dbg = res.results
    return out


# revision 12
# speedup vs baseline: 1.6806x; 1.0114x over previous
"""Cross-attention layer on 8 Trainium2 NeuronCores (Bass/Tile SPMD).

Sharding: tensor-parallel over heads. Each core projects Q^T/K^T/V for its
4 heads (fp8e4 DoubleRow matmuls, fp32 accumulate, weights pre-scaled x64 on
host so they clear the fp8 subnormal range; descale folded into the PSUM
evacuation), runs masked softmax attention in bf16 transposed layout, then a
per-batch AllToAll redistributes ctx^T (fp8, x8 scaled) from head-sharded to
token-sharded so every core runs the fp8 output projection + residual +
LayerNorm for its 128-token slice of each batch. The batch split pipelines
attention(b1) under A2A(b0) and O-proj(b0) under A2A(b1).

Row-sums for softmax: DVE chunk-sum + one ones-matmul + one broadcast
matmul per (b,h,lh) instead of 9 tiny PE matmuls.
"""
import sys

sys.path.insert(0, "/opt/trn_rl_repo")

import numpy as np
import ml_dtypes

import concourse.bacc as bacc
import concourse.mybir as mybir
import concourse.tile as tile
from concourse.bass_utils import run_bass_kernel_spmd

BF16 = ml_dtypes.bfloat16
FP8 = ml_dtypes.float8_e4m3   # matches mybir.dt.float8e4 (max 240)

NCORES = 8
P = 128            # partitions / head dim
H = 4096
KT = H // P        # 32 k-tiles along any H contraction
NT = KT // 2       # 16 DoubleRow k-pair steps
NH = 32
NHL = NH // NCORES  # 4 local heads
CW = NHL * P       # 512 local c-columns
B = 2
LB = 1024          # tokens per batch
L2 = B * LB        # 2048 total tokens
TLB = LB // NCORES  # 128 tokens per core per batch
QW = 512           # token-block width in phase A
NQ = L2 // QW      # 4
MSK = -1e30
WS = 64.0          # fp8 weight pre-scale
CS = 8.0           # fp8 ctx pre-scale
SQ = 1.0 / np.sqrt(P)

_CACHE = {}

F32 = mybir.dt.float32
BF = mybir.dt.bfloat16
F8 = mybir.dt.float8e4
DR = mybir.MatmulPerfMode.DoubleRow


def _build(debug=False):
    nc = bacc.Bacc("TRN2", target_bir_lowering=False, debug=False,
                   num_devices=NCORES)

    hid_d = nc.dram_tensor("hid8", [P, NQ, KT, QW], F8, kind="ExternalInput")
    vis_d = nc.dram_tensor("vis8", [P, NQ, KT, QW], F8, kind="ExternalInput")
    wq_d = nc.dram_tensor("wq8", [P, KT, CW], F8, kind="ExternalInput")
    wk_d = nc.dram_tensor("wk8", [P, KT, CW], F8, kind="ExternalInput")
    wv_d = nc.dram_tensor("wv8", [P, KT, CW], F8, kind="ExternalInput")
    wo_d = nc.dram_tensor("wo8", [P, H // QW, KT, QW], F8, kind="ExternalInput")
    bqT_d = nc.dram_tensor("bqT", [P, NHL], F32, kind="ExternalInput")
    bkT_d = nc.dram_tensor("bkT", [P, NHL], F32, kind="ExternalInput")
    mskb_d = nc.dram_tensor("mskb", [P, B * 8], F32, kind="ExternalInput")
    hb_d = nc.dram_tensor("hb", [B * TLB, H], F32, kind="ExternalInput")
    g_d = nc.dram_tensor("g", [P, H], F32, kind="ExternalInput")
    bta_d = nc.dram_tensor("bta", [P, H], F32, kind="ExternalInput")
    out_d = nc.dram_tensor("out", [B * TLB, H], F32, kind="ExternalOutput")
    if debug:
        qT_dbg = nc.dram_tensor("qT_dbg", [P, NHL * L2], BF, kind="ExternalOutput")
        kT_dbg = nc.dram_tensor("kT_dbg", [P, NHL * L2], BF, kind="ExternalOutput")
        v_dbg = nc.dram_tensor("v_dbg", [P, 16 * CW], BF, kind="ExternalOutput")
        ctxT_dbg = nc.dram_tensor("ctxT_dbg", [P, NHL * L2], F32, kind="ExternalOutput")
        octxT_dbg = nc.dram_tensor("octxT_dbg", [P, B * KT * TLB], F32,
                                   kind="ExternalOutput")
        xpre_dbg = nc.dram_tensor("xpre_dbg", [B * TLB, H], F32, kind="ExternalOutput")

    with tile.TileContext(nc) as tc:
        with tc.tile_pool(name="persist", bufs=1) as pers, \
             tc.tile_pool(name="dram", bufs=1, space="DRAM") as dram:

            pqkv = tc.alloc_tile_pool(name="pqkv", bufs=1)
            qT_sb = pqkv.tile([P, NHL, L2], BF)     # Q^T/sqrt(hd): [hd, h, l]
            kT_sb = pqkv.tile([P, NHL, L2], BF)     # K^T: [hd, h, v]
            v_sb = pqkv.tile([P, 16, CW], BF)       # V: [v, vt, c]
            ctxT_sb = pqkv.tile([P, NHL, L2], F8)   # ctx^T * CS: [hd, h, l]
            bqT_sb = pers.tile([P, NHL], F32)
            bkT_sb = pers.tile([P, NHL], F32)
            mskb_sb = pers.tile([P, B * 8], F32)
            ones_bf = pers.tile([P, 1], BF)
            ones_f32 = pers.tile([1, P], F32)
            nc.sync.dma_start(out=bqT_sb[:], in_=bqT_d[:])
            nc.sync.dma_start(out=bkT_sb[:], in_=bkT_d[:])
            nc.sync.dma_start(out=mskb_sb[:], in_=mskb_d[:])
            nc.vector.memset(ones_bf[:], 1.0)
            nc.vector.memset(ones_f32[:], 1.0)

            # ---------------- Phase A: Q^T, K^T, V projections (fp8 DR) ----
            pa = tc.alloc_tile_pool(name="phaseA", bufs=1)
            with tc.tile_pool(name="psA", bufs=6, space="PSUM") as psA:
                wq_sb = pa.tile([P, KT, CW], F8, name="wq_sb")
                wk_sb = pa.tile([P, KT, CW], F8, name="wk_sb")
                wv_sb = pa.tile([P, KT, CW], F8, name="wv_sb")
                nc.sync.dma_start(out=wq_sb[:], in_=wq_d[:])

                def qk_block(xT, w_sb, b_sb, dst_sb, q, scale):
                    for h in range(NHL):
                        ps = psA.tile([P, QW], F32, tag="psA")
                        for t in range(NT):
                            nc.tensor.matmul(
                                ps[:],
                                w_sb[:, 2 * t:2 * t + 2, h * P:(h + 1) * P],
                                xT[:, 2 * t:2 * t + 2, :],
                                start=(t == 0), stop=(t == NT - 1),
                                perf_mode=DR)
                        nc.vector.tensor_scalar(
                            out=dst_sb[:, h, q * QW:(q + 1) * QW], in0=ps[:],
                            scalar1=scale, scalar2=b_sb[:, h:h + 1],
                            op0=mybir.AluOpType.mult, op1=mybir.AluOpType.add)

                # Q over hid blocks
                for q in range(NQ):
                    xT = pa.tile([P, KT, QW], F8, tag="xT", bufs=3)
                    nc.sync.dma_start(out=xT[:], in_=hid_d[:, q])
                    if q == 0:
                        nc.sync.dma_start(out=wk_sb[:], in_=wk_d[:])
                        nc.sync.dma_start(out=wv_sb[:], in_=wv_d[:])
                    qk_block(xT, wq_sb, bqT_sb, qT_sb, q, SQ / WS)

                # K and V share the vis block loads
                for q in range(NQ):
                    xT = pa.tile([P, KT, QW], F8, tag="xT", bufs=3)
                    nc.sync.dma_start(out=xT[:], in_=vis_d[:, q])
                    qk_block(xT, wk_sb, bkT_sb, kT_sb, q, 1.0 / WS)
                    for vt in range(4):
                        g_vt = q * 4 + vt
                        ps = psA.tile([P, CW], F32, tag="psA")
                        for t in range(NT):
                            nc.tensor.matmul(
                                ps[:],
                                xT[:, 2 * t:2 * t + 2, vt * P:(vt + 1) * P],
                                wv_sb[:, 2 * t:2 * t + 2, :],
                                start=(t == 0), stop=(t == NT - 1),
                                perf_mode=DR)
                        nc.scalar.activation(
                            v_sb[:, g_vt, :], ps[:],
                            mybir.ActivationFunctionType.Copy, scale=1.0 / WS)
            pa.release()

            # ------------- Phase B + C pipelined per batch -----------------
            pb = tc.alloc_tile_pool(name="phaseB", bufs=1)
            pc = tc.alloc_tile_pool(name="phaseC", bufs=1)
            ps = tc.alloc_tile_pool(name="psBC", bufs=1, space="PSUM")

            a2a_in = [dram.tile([NCORES, P, NHL, TLB], F8, name=f"a2a_in{i}")
                      for i in range(B)]
            a2a_out = [dram.tile([NCORES, P, NHL, TLB], F8, name=f"a2a_out{i}")
                       for i in range(B)]
            octxT = [None, None]

            def open_iter(b, h, lh):
                qoff = b * LB + lh * QW
                attnT = pb.tile([P, 8, QW], BF, tag="attnT", bufs=2)
                for vb in range(8):
                    sc_ps = ps.tile([P, QW], F32, tag="sc", bufs=2)
                    nc.tensor.matmul(
                        sc_ps[:],
                        kT_sb[:, h, b * LB + vb * P: b * LB + (vb + 1) * P],
                        qT_sb[:, h, qoff: qoff + QW],
                        start=True, stop=True)
                    nc.scalar.activation(
                        attnT[:, vb, :], sc_ps[:],
                        mybir.ActivationFunctionType.Exp,
                        bias=mskb_sb[:, b * 8 + vb: b * 8 + vb + 1],
                        scale=1.0)
                # chunk-sum on DVE (bf16): S = sum_vb attnT[:, vb]
                S = pb.tile([P, QW], BF, tag="S", bufs=2)
                nc.vector.tensor_tensor(
                    out=S[:], in0=attnT[:, 0, :], in1=attnT[:, 1, :],
                    op=mybir.AluOpType.add)
                for vb in range(2, 8):
                    nc.vector.tensor_tensor(
                        out=S[:], in0=S[:], in1=attnT[:, vb, :],
                        op=mybir.AluOpType.add)
                return (b, h, lh, attnT, S)

            def finish_iter(st):
                b, h, lh, attnT, S = st
                qoff = b * LB + lh * QW
                # cross-partition row-sum + reciprocal + broadcast
                rs_ps = ps.tile([1, QW], F32, tag="rs", bufs=1)
                nc.tensor.matmul(rs_ps[:], ones_bf[:], S[:],
                                 start=True, stop=True)
                rcp_sb = pb.tile([1, QW], F32, tag="rcp", bufs=2)
                nc.vector.reciprocal(rcp_sb[:], rs_ps[:])
                rcp_ps = ps.tile([P, QW], F32, tag="rcpp", bufs=1)
                nc.tensor.matmul(rcp_ps[:], ones_f32[:], rcp_sb[:],
                                 start=True, stop=True)
                rcp_rep = pb.tile([P, QW], F32, tag="rcprep", bufs=2)
                nc.scalar.copy(out=rcp_rep[:], in_=rcp_ps[:])
                ctx_ps = ps.tile([P, QW], F32, tag="ctx", bufs=2)
                for vb in range(8):
                    nc.tensor.matmul(
                        ctx_ps[:],
                        v_sb[:, b * 8 + vb, h * P:(h + 1) * P],
                        attnT[:, vb, :],
                        start=(vb == 0), stop=(vb == 7))
                # normalize * CS -> fp8 ctx^T
                nc.vector.scalar_tensor_tensor(
                    out=ctxT_sb[:, h, qoff: qoff + QW],
                    in0=ctx_ps[:], scalar=CS, in1=rcp_rep[:],
                    op0=mybir.AluOpType.mult, op1=mybir.AluOpType.mult)

            pending = [None]

            def phaseB(b):
                for h in range(NHL):
                    for lh in range(2):
                        st = open_iter(b, h, lh)
                        if pending[0] is not None:
                            finish_iter(pending[0])
                        pending[0] = st

            def a2a(b):
                for h in range(NHL):
                    nc.sync.dma_start(
                        out=a2a_in[b][:, :, h, :].rearrange("j p l -> p j l"),
                        in_=ctxT_sb[:, h, b * LB:(b + 1) * LB]
                            .rearrange("p (j l) -> p j l", j=NCORES))
                nc.gpsimd.collective_compute(
                    "AllToAll", mybir.AluOpType.bypass,
                    replica_groups=[list(range(NCORES))],
                    ins=[a2a_in[b][:]], outs=[a2a_out[b][:]])
                octxT[b] = pc.tile([P, KT, TLB], F8, name=f"octxT{b}")
                for i in range(NCORES):
                    nc.sync.dma_start(
                        out=octxT[b][:, i * NHL:(i + 1) * NHL, :],
                        in_=a2a_out[b][i])

            def phaseC(b, g_sb, bta_sb):
                # O-projection for this batch's 128 tokens, all H columns
                hb_sb = pc.tile([P, H], F32, tag="hb", bufs=1)
                nc.sync.dma_start(out=hb_sb[:], in_=hb_d[b * TLB:(b + 1) * TLB, :])
                x = pc.tile([P, H], F32, tag="x", bufs=2)
                msum = pc.tile([P, 8], F32, tag="msum", bufs=2)
                for mc in range(H // QW):
                    wo_sb = pc.tile([P, KT, QW], F8, tag="wo", bufs=2)
                    nc.sync.dma_start(out=wo_sb[:], in_=wo_d[:, mc])
                    po = ps.tile([P, QW], F32, tag="po", bufs=2)
                    for t in range(NT):
                        nc.tensor.matmul(
                            po[:],
                            octxT[b][:, 2 * t:2 * t + 2, :],
                            wo_sb[:, 2 * t:2 * t + 2, :],
                            start=(t == 0), stop=(t == NT - 1),
                            perf_mode=DR)
                    # x = po/(WS*CS) + hb, with per-chunk row-sum accumulation
                    nc.vector.scalar_tensor_tensor(
                        out=x[:, mc * QW:(mc + 1) * QW],
                        in0=po[:], scalar=1.0 / (WS * CS),
                        in1=hb_sb[:, mc * QW:(mc + 1) * QW],
                        op0=mybir.AluOpType.mult, op1=mybir.AluOpType.add,
                        accum_out=msum[:, mc:mc + 1])
                if debug:
                    nc.sync.dma_start(
                        out=xpre_dbg[b * TLB:(b + 1) * TLB, :], in_=x[:])
                # ---- LayerNorm over H for the 128 tokens ----
                musum = pc.tile([P, 1], F32, tag="musum", bufs=2)
                nc.scalar.activation(
                    msum[:], msum[:], mybir.ActivationFunctionType.Copy,
                    accum_out=musum[:])
                mu = pc.tile([P, 1], F32, tag="mu", bufs=2)
                nc.scalar.mul(mu[:], musum[:], 1.0 / H)
                # Square scratch into hb_sb (already consumed by the evacs)
                ssq = pc.tile([P, 1], F32, tag="ssq", bufs=2)
                nc.scalar.activation(
                    hb_sb[:], x[:], mybir.ActivationFunctionType.Square,
                    accum_out=ssq[:])
                mu2 = pc.tile([P, 1], F32, tag="mu2", bufs=2)
                nc.scalar.activation(mu2[:], mu[:],
                                     mybir.ActivationFunctionType.Square)
                var = pc.tile([P, 1], F32, tag="var", bufs=2)
                nc.vector.scalar_tensor_tensor(
                    out=var[:], in0=ssq[:], scalar=1.0 / H, in1=mu2[:],
                    op0=mybir.AluOpType.mult, op1=mybir.AluOpType.subtract)
                eps_sb = pc.tile([P, 1], F32, tag="eps", bufs=1)
                nc.vector.memset(eps_sb[:], 1e-5)
                std = pc.tile([P, 1], F32, tag="std", bufs=2)
                nc.scalar.activation(std[:], var[:],
                                     mybir.ActivationFunctionType.Sqrt,
                                     bias=eps_sb[:], scale=1.0)
                rstd = pc.tile([P, 1], F32, tag="rstd", bufs=2)
                nc.vector.reciprocal(rstd[:], std[:])
                # normalize + gain + bias, in place on x
                nc.vector.tensor_scalar(
                    out=x[:], in0=x[:], scalar1=mu[:], scalar2=rstd[:],
                    op0=mybir.AluOpType.subtract, op1=mybir.AluOpType.mult)
                nc.vector.tensor_tensor(
                    out=x[:], in0=x[:], in1=g_sb[:], op=mybir.AluOpType.mult)
                nc.gpsimd.tensor_tensor(
                    out=x[:], in0=x[:], in1=bta_sb[:], op=mybir.AluOpType.add)
                nc.sync.dma_start(out=out_d[b * TLB:(b + 1) * TLB, :],
                                  in_=x[:])

            phaseB(0)
            finish_iter(pending[0]); pending[0] = None
            a2a(0)
            phaseB(1)
            finish_iter(pending[0]); pending[0] = None
            if debug:
                nc.sync.dma_start(
                    out=qT_dbg[:], in_=qT_sb[:].rearrange("p h l -> p (h l)"))
                nc.sync.dma_start(
                    out=kT_dbg[:], in_=kT_sb[:].rearrange("p h l -> p (h l)"))
                nc.sync.dma_start(
                    out=v_dbg[:], in_=v_sb[:].rearrange("p t c -> p (t c)"))
                for h in range(NHL):
                    ctmp = pers.tile([P, L2], F32, tag="ctmp", bufs=2)
                    nc.scalar.copy(out=ctmp[:], in_=ctxT_sb[:, h, :])
                    nc.sync.dma_start(
                        out=ctxT_dbg[:, h * L2:(h + 1) * L2], in_=ctmp[:])
            g_sb = pc.tile([P, H], F32, name="g_sb")
            nc.sync.dma_start(out=g_sb[:], in_=g_d[:])
            bta_sb = pc.tile([P, H], F32, name="bta_sb")
            nc.sync.dma_start(out=bta_sb[:], in_=bta_d[:])
            a2a(1)
            phaseC(0, g_sb, bta_sb)
            if debug:
                for b in range(B):
                    for kq in range(4):
                        otmp = pers.tile([P, 8 * TLB], F32, tag="otmp", bufs=2)
                        nc.scalar.copy(
                            out=otmp[:],
                            in_=octxT[b][:, kq * 8:(kq + 1) * 8, :]
                                .rearrange("p k l -> p (k l)"))
                        nc.sync.dma_start(
                            out=octxT_dbg[:, (b * KT + kq * 8) * TLB:
                                          (b * KT + (kq + 1) * 8) * TLB],
                            in_=otmp[:])
            phaseC(1, g_sb, bta_sb)
            ps.release()
            pc.release()
            pb.release()
            pqkv.release()

    nc.compile()
    return nc


def _prep_inputs(hidden_states, vision_features, attention_mask,
                 Wq, bq, Wk, bk, Wv, bv, Wo, bo, ln_g, ln_b):
    f = np.asarray
    hs = f(hidden_states, dtype=np.float32).reshape(L2, H)
    vf = f(vision_features, dtype=np.float32).reshape(L2, H)
    am = f(attention_mask)
    Wq, bq = f(Wq, dtype=np.float32), f(bq, dtype=np.float32)
    Wk, bk = f(Wk, dtype=np.float32), f(bk, dtype=np.float32)
    Wv, bv = f(Wv, dtype=np.float32), f(bv, dtype=np.float32)
    Wo, bo = f(Wo, dtype=np.float32), f(bo, dtype=np.float32)
    ln_g, ln_b = f(ln_g, dtype=np.float32), f(ln_b, dtype=np.float32)

    def act_layout(x):  # [L2, H] -> [P, NQ, KT, QW] fp8
        return np.ascontiguousarray(
            x.T.reshape(KT, P, NQ, QW).transpose(1, 2, 0, 3)).astype(FP8)

    def w_layout(w_slice):  # [CW, H] -> [P, KT, CW] fp8, pre-scaled
        return np.ascontiguousarray(
            (w_slice.T * WS).reshape(KT, P, CW).transpose(1, 0, 2)).astype(FP8)

    hid8 = act_layout(hs)
    vis8 = act_layout(vf)
    # Wo full: [H(c), H(m)] -> [P, H//QW, KT, QW]
    wo8 = np.ascontiguousarray(
        (Wo.T * WS).reshape(KT, P, H // QW, QW).transpose(1, 2, 0, 3)).astype(FP8)
    mb = np.where(am != 0, 0.0, MSK).astype(np.float32)          # (B, LB)
    mskb = np.ascontiguousarray(
        mb.reshape(B, 8, P).transpose(2, 0, 1).reshape(P, B * 8))
    bo_eff = bo + Wo @ bv
    g_rep = np.ascontiguousarray(np.broadcast_to(ln_g, (P, H)))
    b_rep = np.ascontiguousarray(np.broadcast_to(ln_b, (P, H)))

    in_maps = []
    for c in range(NCORES):
        sl = slice(c * CW, (c + 1) * CW)
        hb = np.empty((B * TLB, H), np.float32)
        for b in range(B):
            rows = hs[b * LB + c * TLB:b * LB + (c + 1) * TLB]
            hb[b * TLB:(b + 1) * TLB] = rows + bo_eff
        in_maps.append({
            "hid8": hid8,
            "vis8": vis8,
            "wq8": w_layout(Wq[sl]),
            "wk8": w_layout(Wk[sl]),
            "wv8": w_layout(Wv[sl]),
            "wo8": wo8,
            "bqT": np.ascontiguousarray((bq[sl] * SQ).reshape(NHL, P).T),
            "bkT": np.ascontiguousarray(bk[sl].reshape(NHL, P).T),
            "mskb": mskb,
            "hb": hb,
            "g": g_rep,
            "bta": b_rep,
        })
    return in_maps


def kernel(**inputs) -> np.ndarray:
    key = "dbg" if inputs.pop("_debug", False) else "main"
    if key not in _CACHE:
        _CACHE[key] = _build(debug=(key == "dbg"))
    nc = _CACHE[key]
    in_maps = _prep_inputs(**inputs)
    res = run_bass_kernel_spmd(nc, in_maps, list(range(NCORES)))
    out = np.empty((B, LB, H), np.float32)
    for c in range(NCORES):
        o = res.results[c]["out"]
        for b in range(B):
            out[b, c * TLB:(c + 1) * TLB] = o[b * TLB:(b + 1) * TLB]
    if key == "dbg":
        kernel._dbg = res.results
    return out


# revision 15
# speedup vs baseline: 1.7942x; 1.0676x over previous
"""Cross-attention layer on 8 Trainium2 NeuronCores (Bass/Tile SPMD).

Sharding: tensor-parallel over heads. Each core projects Q^T/K^T/V for its
4 heads (fp8e4 DoubleRow matmuls, fp32 accumulate, weights pre-scaled x64 on
host so they clear the fp8 subnormal range; descale folded into the PSUM
evacuation), runs masked softmax attention in bf16 transposed layout, then a
per-batch AllToAll redistributes ctx^T (fp8, x8 scaled) from head-sharded to
token-sharded so every core runs the fp8 output projection + residual +
LayerNorm for its 128-token slice of each batch. The batch split pipelines
attention(b1) under A2A(b0) and O-proj(b0) under A2A(b1).

Row-sums for softmax: DVE chunk-sum + one ones-matmul + one broadcast
matmul per (b,h,lh) instead of 9 tiny PE matmuls.
"""
import sys

sys.path.insert(0, "/opt/trn_rl_repo")

import numpy as np
import ml_dtypes

import concourse.bacc as bacc
import concourse.mybir as mybir
import concourse.tile as tile
from concourse.bass_utils import run_bass_kernel_spmd

BF16 = ml_dtypes.bfloat16
FP8 = ml_dtypes.float8_e4m3   # matches mybir.dt.float8e4 (max 240)

NCORES = 8
P = 128            # partitions / head dim
H = 4096
KT = H // P        # 32 k-tiles along any H contraction
NT = KT // 2       # 16 DoubleRow k-pair steps
NH = 32
NHL = NH // NCORES  # 4 local heads
CW = NHL * P       # 512 local c-columns
B = 2
LB = 1024          # tokens per batch
L2 = B * LB        # 2048 total tokens
TLB = LB // NCORES  # 128 tokens per core per batch
QW = 512           # token-block width in phase A
NQ = L2 // QW      # 4
MSK = -1e30
WS = 64.0          # fp8 weight pre-scale
CS = 8.0           # fp8 ctx pre-scale
SQ = 1.0 / np.sqrt(P)

_CACHE = {}

F32 = mybir.dt.float32
BF = mybir.dt.bfloat16
F8 = mybir.dt.float8e4
DR = mybir.MatmulPerfMode.DoubleRow


def _build(debug=False):
    nc = bacc.Bacc("TRN2", target_bir_lowering=False, debug=False,
                   num_devices=NCORES)

    hid_d = nc.dram_tensor("hid8", [P, NQ, KT, QW], F8, kind="ExternalInput")
    vis_d = nc.dram_tensor("vis8", [P, NQ, KT, QW], F8, kind="ExternalInput")
    wq_d = nc.dram_tensor("wq8", [P, KT, CW], F8, kind="ExternalInput")
    wk_d = nc.dram_tensor("wk8", [P, KT, CW], F8, kind="ExternalInput")
    wv_d = nc.dram_tensor("wv8", [P, KT, CW], F8, kind="ExternalInput")
    wo_d = nc.dram_tensor("wo8", [P, H // QW, KT, QW], F8, kind="ExternalInput")
    bqT_d = nc.dram_tensor("bqT", [P, NHL], F32, kind="ExternalInput")
    bkT_d = nc.dram_tensor("bkT", [P, NHL], F32, kind="ExternalInput")
    mskb_d = nc.dram_tensor("mskb", [P, B * 8], F32, kind="ExternalInput")
    hb_d = nc.dram_tensor("hb", [B * TLB, H], F32, kind="ExternalInput")
    g_d = nc.dram_tensor("g", [P, H], F32, kind="ExternalInput")
    bta_d = nc.dram_tensor("bta", [P, H], F32, kind="ExternalInput")
    out_d = nc.dram_tensor("out", [B * TLB, H], F32, kind="ExternalOutput")
    if debug:
        qT_dbg = nc.dram_tensor("qT_dbg", [P, NHL * L2], BF, kind="ExternalOutput")
        kT_dbg = nc.dram_tensor("kT_dbg", [P, NHL * L2], BF, kind="ExternalOutput")
        v_dbg = nc.dram_tensor("v_dbg", [P, 16 * CW], BF, kind="ExternalOutput")
        ctxT_dbg = nc.dram_tensor("ctxT_dbg", [P, NHL * L2], F32, kind="ExternalOutput")
        octxT_dbg = nc.dram_tensor("octxT_dbg", [P, B * KT * TLB], F32,
                                   kind="ExternalOutput")
        xpre_dbg = nc.dram_tensor("xpre_dbg", [B * TLB, H], F32, kind="ExternalOutput")

    with tile.TileContext(nc) as tc:
        with tc.tile_pool(name="persist", bufs=1) as pers, \
             tc.tile_pool(name="dram", bufs=1, space="DRAM") as dram:

            pqkv = tc.alloc_tile_pool(name="pqkv", bufs=1)
            qT_sb = pqkv.tile([P, NHL, L2], BF)     # Q^T/sqrt(hd): [hd, h, l]
            kT_sb = pqkv.tile([P, NHL, L2], BF)     # K^T: [hd, h, v]
            v_sb = pqkv.tile([P, 16, CW], BF)       # V: [v, vt, c]
            ctxT_sb = pqkv.tile([P, NHL, L2], F8)   # ctx^T * CS: [hd, h, l]
            bqT_sb = pers.tile([P, NHL], F32)
            bkT_sb = pers.tile([P, NHL], F32)
            mskb_sb = pers.tile([P, B * 8], F32)
            ones_bf = pers.tile([P, 1], BF)
            ones_f32 = pers.tile([1, P], F32)
            nc.sync.dma_start(out=bqT_sb[:], in_=bqT_d[:])
            nc.sync.dma_start(out=bkT_sb[:], in_=bkT_d[:])
            nc.sync.dma_start(out=mskb_sb[:], in_=mskb_d[:])
            nc.vector.memset(ones_bf[:], 1.0)
            nc.vector.memset(ones_f32[:], 1.0)

            # ---------------- Phase A: Q^T, K^T, V projections (fp8 DR) ----
            pa = tc.alloc_tile_pool(name="phaseA", bufs=1)
            with tc.tile_pool(name="psA", bufs=6, space="PSUM") as psA:
                wq_sb = pa.tile([P, KT, CW], F8, name="wq_sb")
                wk_sb = pa.tile([P, KT, CW], F8, name="wk_sb")
                wv_sb = pa.tile([P, KT, CW], F8, name="wv_sb")
                nc.sync.dma_start(out=wq_sb[:, :KT // 2, :], in_=wq_d[:, :KT // 2, :])
                nc.sync.dma_start(out=wq_sb[:, KT // 2:, :], in_=wq_d[:, KT // 2:, :])

                def qk_block(xT, w_sb, b_sb, dst_sb, q, scale):
                    for h in range(NHL):
                        ps = psA.tile([P, QW], F32, tag="psA")
                        for t in range(NT):
                            nc.tensor.matmul(
                                ps[:],
                                w_sb[:, 2 * t:2 * t + 2, h * P:(h + 1) * P],
                                xT[:, 2 * t:2 * t + 2, :],
                                start=(t == 0), stop=(t == NT - 1),
                                perf_mode=DR)
                        nc.vector.tensor_scalar(
                            out=dst_sb[:, h, q * QW:(q + 1) * QW], in0=ps[:],
                            scalar1=scale, scalar2=b_sb[:, h:h + 1],
                            op0=mybir.AluOpType.mult, op1=mybir.AluOpType.add)

                # Q over hid blocks
                for q in range(NQ):
                    xT = pa.tile([P, KT, QW], F8, tag="xT", bufs=3)
                    nc.sync.dma_start(out=xT[:], in_=hid_d[:, q])
                    if q == 0:
                        nc.sync.dma_start(out=wk_sb[:], in_=wk_d[:])
                        nc.sync.dma_start(out=wv_sb[:], in_=wv_d[:])
                    qk_block(xT, wq_sb, bqT_sb, qT_sb, q, SQ / WS)

                # K and V share the vis block loads
                for q in range(NQ):
                    xT = pa.tile([P, KT, QW], F8, tag="xT", bufs=3)
                    nc.sync.dma_start(out=xT[:], in_=vis_d[:, q])
                    qk_block(xT, wk_sb, bkT_sb, kT_sb, q, 1.0 / WS)
                    for vt in range(4):
                        g_vt = q * 4 + vt
                        ps = psA.tile([P, CW], F32, tag="psA")
                        for t in range(NT):
                            nc.tensor.matmul(
                                ps[:],
                                xT[:, 2 * t:2 * t + 2, vt * P:(vt + 1) * P],
                                wv_sb[:, 2 * t:2 * t + 2, :],
                                start=(t == 0), stop=(t == NT - 1),
                                perf_mode=DR)
                        nc.scalar.activation(
                            v_sb[:, g_vt, :], ps[:],
                            mybir.ActivationFunctionType.Copy, scale=1.0 / WS)
            pa.release()

            # ------------- Phase B + C pipelined per batch -----------------
            pb = tc.alloc_tile_pool(name="phaseB", bufs=1)
            pc = tc.alloc_tile_pool(name="phaseC", bufs=1)
            ps = tc.alloc_tile_pool(name="psBC", bufs=1, space="PSUM")

            a2a_in = [dram.tile([NCORES, P, NHL, TLB], F8, name=f"a2a_in{i}")
                      for i in range(B)]
            a2a_out = [dram.tile([NCORES, P, NHL, TLB], F8, name=f"a2a_out{i}")
                       for i in range(B)]
            octxT = [None, None]

            def open_iter(b, h, lh):
                qoff = b * LB + lh * QW
                attnT = pb.tile([P, 8, QW], BF, tag="attnT", bufs=2)
                for vb in range(8):
                    sc_ps = ps.tile([P, QW], F32, tag="sc", bufs=2)
                    nc.tensor.matmul(
                        sc_ps[:],
                        kT_sb[:, h, b * LB + vb * P: b * LB + (vb + 1) * P],
                        qT_sb[:, h, qoff: qoff + QW],
                        start=True, stop=True)
                    nc.scalar.activation(
                        attnT[:, vb, :], sc_ps[:],
                        mybir.ActivationFunctionType.Exp,
                        bias=mskb_sb[:, b * 8 + vb: b * 8 + vb + 1],
                        scale=1.0)
                # chunk-sum on DVE (bf16): S = sum_vb attnT[:, vb]
                S = pb.tile([P, QW], BF, tag="S", bufs=2)
                nc.vector.tensor_tensor(
                    out=S[:], in0=attnT[:, 0, :], in1=attnT[:, 1, :],
                    op=mybir.AluOpType.add)
                for vb in range(2, 8):
                    nc.vector.tensor_tensor(
                        out=S[:], in0=S[:], in1=attnT[:, vb, :],
                        op=mybir.AluOpType.add)
                return (b, h, lh, attnT, S)

            def finish_iter(st):
                b, h, lh, attnT, S = st
                qoff = b * LB + lh * QW
                # cross-partition row-sum + reciprocal + broadcast
                rs_ps = ps.tile([1, QW], F32, tag="rs", bufs=1)
                nc.tensor.matmul(rs_ps[:], ones_bf[:], S[:],
                                 start=True, stop=True)
                rcp_sb = pb.tile([1, QW], F32, tag="rcp", bufs=2)
                nc.vector.reciprocal_approx_fast(out=rcp_sb[:], in_=rs_ps[:])
                rcp_ps = ps.tile([P, QW], F32, tag="rcpp", bufs=1)
                nc.tensor.matmul(rcp_ps[:], ones_f32[:], rcp_sb[:],
                                 start=True, stop=True)
                rcp_rep = pb.tile([P, QW], F32, tag="rcprep", bufs=2)
                nc.scalar.copy(out=rcp_rep[:], in_=rcp_ps[:])
                ctx_ps = ps.tile([P, QW], F32, tag="ctx", bufs=2)
                for vb in range(8):
                    nc.tensor.matmul(
                        ctx_ps[:],
                        v_sb[:, b * 8 + vb, h * P:(h + 1) * P],
                        attnT[:, vb, :],
                        start=(vb == 0), stop=(vb == 7))
                # normalize * CS -> fp8 ctx^T
                nc.vector.scalar_tensor_tensor(
                    out=ctxT_sb[:, h, qoff: qoff + QW],
                    in0=ctx_ps[:], scalar=CS, in1=rcp_rep[:],
                    op0=mybir.AluOpType.mult, op1=mybir.AluOpType.mult)

            pending = [None]

            def phaseB(b):
                for h in range(NHL):
                    for lh in range(2):
                        st = open_iter(b, h, lh)
                        if pending[0] is not None:
                            finish_iter(pending[0])
                        pending[0] = st

            def a2a(b):
                for h in range(NHL):
                    nc.sync.dma_start(
                        out=a2a_in[b][:, :, h, :].rearrange("j p l -> p j l"),
                        in_=ctxT_sb[:, h, b * LB:(b + 1) * LB]
                            .rearrange("p (j l) -> p j l", j=NCORES))
                nc.gpsimd.collective_compute(
                    "AllToAll", mybir.AluOpType.bypass,
                    replica_groups=[list(range(NCORES))],
                    ins=[a2a_in[b][:]], outs=[a2a_out[b][:]])
                octxT[b] = pc.tile([P, KT, TLB], F8, name=f"octxT{b}")
                for i in range(NCORES):
                    nc.sync.dma_start(
                        out=octxT[b][:, i * NHL:(i + 1) * NHL, :],
                        in_=a2a_out[b][i])

            def phaseC(b, g_sb, bta_sb):
                # O-projection for this batch's 128 tokens, all H columns
                hb_sb = pc.tile([P, H], F32, tag="hb", bufs=1)
                nc.sync.dma_start(out=hb_sb[:], in_=hb_d[b * TLB:(b + 1) * TLB, :])
                x = pc.tile([P, H], F32, tag="x", bufs=2)
                msum = pc.tile([P, 8], F32, tag="msum", bufs=2)
                for mc in range(H // QW):
                    wo_sb = pc.tile([P, KT, QW], F8, tag="wo", bufs=2)
                    nc.sync.dma_start(out=wo_sb[:], in_=wo_d[:, mc])
                    po = ps.tile([P, QW], F32, tag="po", bufs=2)
                    for t in range(NT):
                        nc.tensor.matmul(
                            po[:],
                            octxT[b][:, 2 * t:2 * t + 2, :],
                            wo_sb[:, 2 * t:2 * t + 2, :],
                            start=(t == 0), stop=(t == NT - 1),
                            perf_mode=DR)
                    # x = po/(WS*CS) + hb, with per-chunk row-sum accumulation
                    nc.vector.scalar_tensor_tensor(
                        out=x[:, mc * QW:(mc + 1) * QW],
                        in0=po[:], scalar=1.0 / (WS * CS),
                        in1=hb_sb[:, mc * QW:(mc + 1) * QW],
                        op0=mybir.AluOpType.mult, op1=mybir.AluOpType.add,
                        accum_out=msum[:, mc:mc + 1])
                if debug:
                    nc.sync.dma_start(
                        out=xpre_dbg[b * TLB:(b + 1) * TLB, :], in_=x[:])
                # ---- LayerNorm over H for the 128 tokens ----
                musum = pc.tile([P, 1], F32, tag="musum", bufs=2)
                nc.scalar.activation(
                    msum[:], msum[:], mybir.ActivationFunctionType.Copy,
                    accum_out=musum[:])
                mu = pc.tile([P, 1], F32, tag="mu", bufs=2)
                nc.scalar.mul(mu[:], musum[:], 1.0 / H)
                # Square scratch into hb_sb (already consumed by the evacs)
                ssq = pc.tile([P, 1], F32, tag="ssq", bufs=2)
                nc.scalar.activation(
                    hb_sb[:], x[:], mybir.ActivationFunctionType.Square,
                    accum_out=ssq[:])
                mu2 = pc.tile([P, 1], F32, tag="mu2", bufs=2)
                nc.scalar.activation(mu2[:], mu[:],
                                     mybir.ActivationFunctionType.Square)
                var = pc.tile([P, 1], F32, tag="var", bufs=2)
                nc.vector.scalar_tensor_tensor(
                    out=var[:], in0=ssq[:], scalar=1.0 / H, in1=mu2[:],
                    op0=mybir.AluOpType.mult, op1=mybir.AluOpType.subtract)
                eps_sb = pc.tile([P, 1], F32, tag="eps", bufs=1)
                nc.vector.memset(eps_sb[:], 1e-5)
                std = pc.tile([P, 1], F32, tag="std", bufs=2)
                nc.scalar.activation(std[:], var[:],
                                     mybir.ActivationFunctionType.Sqrt,
                                     bias=eps_sb[:], scale=1.0)
                rstd = pc.tile([P, 1], F32, tag="rstd", bufs=2)
                nc.vector.reciprocal(rstd[:], std[:])
                # normalize + gain + bias, in place on x
                nc.vector.tensor_scalar(
                    out=x[:], in0=x[:], scalar1=mu[:], scalar2=rstd[:],
                    op0=mybir.AluOpType.subtract, op1=mybir.AluOpType.mult)
                nc.vector.tensor_tensor(
                    out=x[:], in0=x[:], in1=g_sb[:], op=mybir.AluOpType.mult)
                nc.vector.tensor_tensor(
                    out=x[:], in0=x[:], in1=bta_sb[:], op=mybir.AluOpType.add)
                nc.sync.dma_start(out=out_d[b * TLB:(b + 1) * TLB, :],
                                  in_=x[:])

            # g/bta: load one row each (staged through the hb-tag buffer),
            # broadcast on the (idle) GpSimd engine
            g_sb = pc.tile([P, H], F32, name="g_sb")
            bta_sb = pc.tile([P, H], F32, name="bta_sb")
            stage = pc.tile([P, H], F32, tag="hb", bufs=1)
            nc.sync.dma_start(out=stage[0:1, :], in_=g_d[0:1, :])
            nc.gpsimd.partition_broadcast(g_sb[:], stage[0:1, :])
            nc.sync.dma_start(out=stage[0:1, :], in_=bta_d[0:1, :])
            nc.gpsimd.partition_broadcast(bta_sb[:], stage[0:1, :])

            phaseB(0)
            finish_iter(pending[0]); pending[0] = None
            a2a(0)
            phaseB(1)
            finish_iter(pending[0]); pending[0] = None
            if debug:
                nc.sync.dma_start(
                    out=qT_dbg[:], in_=qT_sb[:].rearrange("p h l -> p (h l)"))
                nc.sync.dma_start(
                    out=kT_dbg[:], in_=kT_sb[:].rearrange("p h l -> p (h l)"))
                nc.sync.dma_start(
                    out=v_dbg[:], in_=v_sb[:].rearrange("p t c -> p (t c)"))
                for h in range(NHL):
                    ctmp = pers.tile([P, L2], F32, tag="ctmp", bufs=2)
                    nc.scalar.copy(out=ctmp[:], in_=ctxT_sb[:, h, :])
                    nc.sync.dma_start(
                        out=ctxT_dbg[:, h * L2:(h + 1) * L2], in_=ctmp[:])
            a2a(1)
            phaseC(0, g_sb, bta_sb)
            if debug:
                for b in range(B):
                    for kq in range(4):
                        otmp = pers.tile([P, 8 * TLB], F32, tag="otmp", bufs=2)
                        nc.scalar.copy(
                            out=otmp[:],
                            in_=octxT[b][:, kq * 8:(kq + 1) * 8, :]
                                .rearrange("p k l -> p (k l)"))
                        nc.sync.dma_start(
                            out=octxT_dbg[:, (b * KT + kq * 8) * TLB:
                                          (b * KT + (kq + 1) * 8) * TLB],
                            in_=otmp[:])
            phaseC(1, g_sb, bta_sb)
            ps.release()
            pc.release()
            pb.release()
            pqkv.release()

    nc.compile()
    return nc


def _prep_inputs(hidden_states, vision_features, attention_mask,
                 Wq, bq, Wk, bk, Wv, bv, Wo, bo, ln_g, ln_b):
    f = np.asarray
    hs = f(hidden_states, dtype=np.float32).reshape(L2, H)
    vf = f(vision_features, dtype=np.float32).reshape(L2, H)
    am = f(attention_mask)
    Wq, bq = f(Wq, dtype=np.float32), f(bq, dtype=np.float32)
    Wk, bk = f(Wk, dtype=np.float32), f(bk, dtype=np.float32)
    Wv, bv = f(Wv, dtype=np.float32), f(bv, dtype=np.float32)
    Wo, bo = f(Wo, dtype=np.float32), f(bo, dtype=np.float32)
    ln_g, ln_b = f(ln_g, dtype=np.float32), f(ln_b, dtype=np.float32)

    def act_layout(x):  # [L2, H] -> [P, NQ, KT, QW] fp8
        return np.ascontiguousarray(
            x.T.reshape(KT, P, NQ, QW).transpose(1, 2, 0, 3)).astype(FP8)

    def w_layout(w_slice):  # [CW, H] -> [P, KT, CW] fp8, pre-scaled
        return np.ascontiguousarray(
            (w_slice.T * WS).reshape(KT, P, CW).transpose(1, 0, 2)).astype(FP8)

    hid8 = act_layout(hs)
    vis8 = act_layout(vf)
    # Wo full: [H(c), H(m)] -> [P, H//QW, KT, QW]
    wo8 = np.ascontiguousarray(
        (Wo.T * WS).reshape(KT, P, H // QW, QW).transpose(1, 2, 0, 3)).astype(FP8)
    mb = np.where(am != 0, 0.0, MSK).astype(np.float32)          # (B, LB)
    mskb = np.ascontiguousarray(
        mb.reshape(B, 8, P).transpose(2, 0, 1).reshape(P, B * 8))
    bo_eff = bo + Wo @ bv
    g_rep = np.ascontiguousarray(np.broadcast_to(ln_g, (P, H)))
    b_rep = np.ascontiguousarray(np.broadcast_to(ln_b, (P, H)))

    in_maps = []
    for c in range(NCORES):
        sl = slice(c * CW, (c + 1) * CW)
        hb = np.empty((B * TLB, H), np.float32)
        for b in range(B):
            rows = hs[b * LB + c * TLB:b * LB + (c + 1) * TLB]
            hb[b * TLB:(b + 1) * TLB] = rows + bo_eff
        in_maps.append({
            "hid8": hid8,
            "vis8": vis8,
            "wq8": w_layout(Wq[sl]),
            "wk8": w_layout(Wk[sl]),
            "wv8": w_layout(Wv[sl]),
            "wo8": wo8,
            "bqT": np.ascontiguousarray((bq[sl] * SQ).reshape(NHL, P).T),
            "bkT": np.ascontiguousarray(bk[sl].reshape(NHL, P).T),
            "mskb": mskb,
            "hb": hb,
            "g": g_rep,
            "bta": b_rep,
        })
    return in_maps


def kernel(**inputs) -> np.ndarray:
    key = "dbg" if inputs.pop("_debug", False) else "main"
    if key not in _CACHE:
        _CACHE[key] = _build(debug=(key == "dbg"))
    nc = _CACHE[key]
    in_maps = _prep_inputs(**inputs)
    res = run_bass_kernel_spmd(nc, in_maps, list(range(NCORES)))
    out = np.empty((B, LB, H), np.float32)
    for c in range(NCORES):
        o = res.results[c]["out"]
        for b in range(B):
            out[b, c * TLB:(c + 1) * TLB] = o[b * TLB:(b + 1) * TLB]
    if key == "dbg":
        kernel._dbg = res.results
    return out
